# revision 2
# baseline (speedup 1.0000x reference)
"""Banded-Toeplitz HRF stack kernel for Trainium2 (8 NeuronCores, data-parallel).

Problem: theta [512,1] -> H [512,400,400] f32 where
  k[b,:] = gamma_pdf(t, 5, theta_b) - 0.167 * gamma_pdf(t, 15, theta_b)   (30 taps)
  H[b, j, i] = k[b, j-i] if 0 <= j-i < 30 else 0

Strategy (per core, 64 batches):
  * The 29 taps k[1..29] per batch are computed host-side (512 x 29 values,
    negligible) and staged in the per-core DRAM input as [64, 64] f32 rows
    [krev' (29) | zeros (35)], krev'[q] = k[29-q].  k[0] is ~1e-43 in the
    reference (t is clipped at 1e-8, so t^5 underflows f32) -- the diagonal is
    left unwritten, matching the pre-zeroed output to ~1e-43 absolute.
  * The device program is three dependency-free DRAM->DRAM DMAs that write
    only the in-band elements; everything off-band stays zero because
    run_bass_kernel_spmd pre-zeroes ExternalOutput buffers (donated zero
    buffers under the PJRT/axon path -- documented, test-covered semantics).
    Flat offset of row j's band start is 401j - 29, so:

      Rect BC rows  29-399 cols [j-29, j)  src q = 0..28 (mid step  0)
      Rect A1 rows   1- 19 cols [0, 19)    src q = 29-j+i (mid step -1)
      Rect A2 rows  20- 28 cols [0, 29)    src q = 29-j+i (mid step -1)

    A1/A2 read past the taps into the zero margin for i >= j, writing zeros
    (harmless).  A1 is split off because a 19-element (76 B) descriptor hits
    the 7 ns DMA_MIN_TRANSFER_TIME floor instead of the 116-B rate.

  Cost model: 23744 desc @ 10.31 ns (BC) + 1216 @ 7 (A1) + 576 @ 10.31 (A2)
  over 16 engines = ~16.2 us transfer + ~1.3 us HWDGE/DGE prefix + ~0.9 us
  sem tail.  No SBUF, no compute, no inter-engine sync.
"""

import numpy as np

B = 512
T = 400
L = 30
NCORES = 8
BPC = B // NCORES  # 64 batches per core
IW = 64            # input row width per batch (29 taps + zero margin)

_CACHE = {}


def _host_taps(theta):
    """theta [B] -> krev' [B, 29] f32: krev'[b, q] = k[b, 29-q], taps d=1..29.

    t matches the reference grid: f32 linspace(0, 30, 30000)[::1000], clipped
    at 1e-8.  Tap math in float64 then cast (diff vs the reference's f32
    device math is ~1e-7 relative).
    """
    t = np.linspace(0.0, 30.0, 30000, dtype=np.float32)[::1000]
    t = np.maximum(t, np.float32(1e-8)).astype(np.float64)  # [30]
    b = theta.astype(np.float64)[:, None]                   # [B,1]
    ebt = np.exp(-b * t)
    peak = b**6 * t**5 * ebt / 120.0
    under = b**16 * t**15 * ebt / 1307674368000.0
    k = peak - 0.167 * under                                # [B,30]
    return np.ascontiguousarray(k[:, 29:0:-1]).astype(np.float32)


def _in_map(theta_slice):
    row = np.zeros((BPC, IW), dtype=np.float32)
    row[:, :29] = _host_taps(theta_slice)
    return {"inp": row}


def _build_nc():
    import concourse.bass as bass
    import concourse.mybir as mybir
    from concourse.ap import AP
    from contextlib import ExitStack

    f32 = mybir.dt.float32
    nc = bass.Bass()

    inp = nc.declare_dram_parameter("inp", [BPC, IW], f32, isOutput=False)
    out = nc.declare_dram_parameter("H", [BPC, T, T], f32, isOutput=True)
    in_t = inp[:].tensor
    out_t = out[:].tensor

    ctx = ExitStack()
    nc._kernel_ctx = ctx
    osem = ctx.enter_context(nc.semaphore("osem"))

    # Rect BC: rows 29..399, band cols [j-29, j), fixed 29-tap source.
    bc_src = AP(tensor=in_t, offset=0, ap=[[IW, BPC], [0, 371], [1, 29]])
    bc_dst = AP(tensor=out_t, offset=401 * 29 - 29,
                ap=[[T * T, BPC], [401, 371], [1, 29]])
    # Rect A1: rows 1..19, cols [0,19), sliding window (srcs 29-j .. 47-j).
    a1_src = AP(tensor=in_t, offset=28, ap=[[IW, BPC], [-1, 19], [1, 19]])
    a1_dst = AP(tensor=out_t, offset=T,
                ap=[[T * T, BPC], [T, 19], [1, 19]])
    # Rect A2: rows 20..28, cols [0,29), sliding window (srcs 29-j .. 57-j).
    a2_src = AP(tensor=in_t, offset=9, ap=[[IW, BPC], [-1, 9], [1, 29]])
    a2_dst = AP(tensor=out_t, offset=T * 20,
                ap=[[T * T, BPC], [T, 9], [1, 29]])

    with nc.Block() as block:

        @block.sync
        def _(sync):
            sync.dma_start(bc_dst, bc_src).then_inc(osem, 16)
            sync.dma_start(a1_dst, a1_src).then_inc(osem, 16)
            sync.dma_start(a2_dst, a2_src).then_inc(osem, 16)
            sync.wait_ge(osem, 48)

    return nc


def _get_nc():
    if "nc" not in _CACHE:
        _CACHE["nc"] = _build_nc()
    return _CACHE["nc"]


def kernel(theta):
    from concourse.bass_utils import run_bass_kernel_spmd

    theta = np.asarray(theta, dtype=np.float32).reshape(B)
    in_maps = [_in_map(theta[c * BPC:(c + 1) * BPC]) for c in range(NCORES)]
    nc = _get_nc()
    res = run_bass_kernel_spmd(nc, in_maps, list(range(NCORES)))
    return np.concatenate([res.results[i]["H"] for i in range(NCORES)], axis=0)


# revision 21
# speedup vs baseline: 8.0911x; 8.0911x over previous
"""Banded-Toeplitz HRF stack kernel for Trainium2 (8 NeuronCores, data-parallel).

Problem: theta [512,1] -> H [512,400,400] f32 where
  k[b,:] = gamma_pdf(t, 5, theta_b) - 0.167 * gamma_pdf(t, 15, theta_b)   (30 taps)
  H[b, j, i] = k[b, j-i] if 0 <= j-i < 30 else 0

Strategy (per core, 64 batches):
  * The 29 taps k[1..29] per batch are computed host-side (512 x 29 values,
    negligible) and staged in the per-core DRAM input as [64, 64] f32 rows
    [krev' (29) | zeros (35)], krev'[q] = k[29-q].  k[0] is ~1e-43 in the
    reference (t is clipped at 1e-8, so t^5 underflows f32) -- the diagonal is
    left unwritten, matching the pre-zeroed output to ~1e-43 absolute.
  * The device program writes only the in-band elements with DRAM->DRAM DMAs;
    everything off-band stays zero because run_bass_kernel_spmd pre-zeroes
    ExternalOutput buffers (donated zero buffers under the PJRT/axon path --
    documented, test-covered semantics).  Flat offset of row j's band start is
    401j - 29, so two rect shapes cover the band:

      Rect BC rows  29-399 cols [j-29, j)  src q = 0..28 (fixed window)
      Rect A  rows   1- 28 cols [0, ...)   src q = 29-j+i (sliding window)

    A reads past the taps into the zero margin for i >= j, writing zeros
    (harmless), and is cut into a 3-step staircase -- rows 1-13 @ 13 cols,
    14-21 @ 21, 22-28 @ 28 -- since row j only needs cols [0, j-1].  BC puts
    the 371-row dim first in the access pattern and A puts the 64-batch dim
    first (descriptor sets on hardware are identical under any dim order;
    the leading dim rides the 16-way DMA-engine parallelism).  Work is
    split across the three DMA-capable engines -- SP: BC x16 + A[1..13] +
    A[14..21], Act: BC x21 + A[22..28], Pool: BC x14 + BC x13 -- three
    concurrent DMA pipes balanced against each engine's DMA completion
    latency (SP/Act ~1.72 us, Pool ~1.88 us).  Pool's software DGE cannot
    generate negative-stride descriptors and tops out between 6k and 8.9k
    descriptors per instruction, hence Pool takes only fixed-window BC work
    in 5194/4823-descriptor chunks.
"""

import numpy as np

B = 512
T = 400
L = 30
NCORES = 8
BPC = B // NCORES  # 64 batches per core
IW = 64            # input row width per batch (29 taps + zero margin)

_CACHE = {}


def _host_taps(theta):
    """theta [B] -> krev' [B, 29] f32: krev'[b, q] = k[b, 29-q], taps d=1..29.

    t matches the reference grid: f32 linspace(0, 30, 30000)[::1000], clipped
    at 1e-8.  Tap math in float64 then cast (diff vs the reference's f32
    device math is ~1e-7 relative).
    """
    t = np.linspace(0.0, 30.0, 30000, dtype=np.float32)[::1000]
    t = np.maximum(t, np.float32(1e-8)).astype(np.float64)  # [30]
    b = theta.astype(np.float64)[:, None]                   # [B,1]
    ebt = np.exp(-b * t)
    peak = b**6 * t**5 * ebt / 120.0
    under = b**16 * t**15 * ebt / 1307674368000.0
    k = peak - 0.167 * under                                # [B,30]
    return np.ascontiguousarray(k[:, 29:0:-1]).astype(np.float32)


def _in_map(theta_slice):
    row = np.zeros((BPC, IW), dtype=np.float32)
    row[:, :29] = _host_taps(theta_slice)
    return {"inp": row}


# BC batch split across the three DMA-capable engines.  DMA completion
# (sem update or drain) lands at proc_end + init_delay (1717 ns for SP/Act,
# 1883 for Pool), so the split equalizes proc + init_delay per engine.
# Pool's SWDGE cannot generate negative-stride descriptors, so it takes
# only BC work (strides >= 0); the sliding-window A rects go to SP/Act.
# SWDGE also fails above ~6-8k descriptors per instruction, so Pool's 27
# batches are issued as 14+13-batch DMAs (5194/4823 descriptors).
_SPLIT = [(0, 16), (16, 21), (37, 14), (51, 13)]
# A staircase: (j0, nrows, ncols) pieces; row j only needs cols [0, j-1],
# so narrower columns for lower rows (each piece stays above the 500 ns
# per-instruction floor).  SP takes the first two, Act the third.
_ASPLIT = [(1, 13, 13), (14, 8, 21), (22, 7, 28)]


def _build_nc():
    import concourse.bass as bass
    import concourse.mybir as mybir
    from concourse.ap import AP
    from contextlib import ExitStack

    f32 = mybir.dt.float32
    nc = bass.Bass()

    inp = nc.declare_dram_parameter("inp", [BPC, IW], f32, isOutput=False)
    out = nc.declare_dram_parameter("H", [BPC, T, T], f32, isOutput=True)
    in_t = inp[:].tensor
    out_t = out[:].tensor

    ctx = ExitStack()
    nc._kernel_ctx = ctx
    osem = ctx.enter_context(nc.semaphore("osem"))
    psem = ctx.enter_context(nc.semaphore("psem"))

    def bc_aps(b0, nb):
        src = AP(tensor=in_t, offset=IW * b0,
                 ap=[[0, 371], [IW, nb], [1, 29]])
        dst = AP(tensor=out_t, offset=401 * 29 - 29 + T * T * b0,
                 ap=[[401, 371], [T * T, nb], [1, 29]])
        return dst, src

    def a_aps(j0, nr, ncol):
        # rows j0..j0+nr-1, cols [0, ncol), all 64 batches (batch dim first)
        src = AP(tensor=in_t, offset=29 - j0,
                 ap=[[IW, BPC], [-1, nr], [1, ncol]])
        dst = AP(tensor=out_t, offset=T * j0,
                 ap=[[T * T, BPC], [T, nr], [1, ncol]])
        return dst, src

    with nc.Block() as block:

        @block.sync
        def _(sync):
            sync.dma_start(*bc_aps(*_SPLIT[0])).then_inc(osem, 16)
            sync.dma_start(*a_aps(*_ASPLIT[0])).then_inc(osem, 16)
            sync.dma_start(*a_aps(*_ASPLIT[1])).then_inc(osem, 16)
            sync.wait_ge(osem, 80)
            sync.wait_ge(psem, 32)

        @block.scalar
        def _(scalar):
            scalar.dma_start(*bc_aps(*_SPLIT[1])).then_inc(osem, 16)
            scalar.dma_start(*a_aps(*_ASPLIT[2])).then_inc(osem, 16)

        @block.gpsimd
        def _(gpsimd):
            gpsimd.dma_start(*bc_aps(*_SPLIT[2])).then_inc(psem, 16)
            gpsimd.dma_start(*bc_aps(*_SPLIT[3])).then_inc(psem, 16)

    return nc


def _get_nc():
    if "nc" not in _CACHE:
        _CACHE["nc"] = _build_nc()
    return _CACHE["nc"]


def kernel(theta):
    from concourse.bass_utils import run_bass_kernel_spmd

    theta = np.asarray(theta, dtype=np.float32).reshape(B)
    in_maps = [_in_map(theta[c * BPC:(c + 1) * BPC]) for c in range(NCORES)]
    nc = _get_nc()
    res = run_bass_kernel_spmd(nc, in_maps, list(range(NCORES)))
    return np.concatenate([res.results[i]["H"] for i in range(NCORES)], axis=0)


# revision 22
# speedup vs baseline: 8.4589x; 1.0455x over previous
"""Banded-Toeplitz HRF stack kernel for Trainium2 (8 NeuronCores, data-parallel).

Problem: theta [512,1] -> H [512,400,400] f32 where
  k[b,:] = gamma_pdf(t, 5, theta_b) - 0.167 * gamma_pdf(t, 15, theta_b)   (30 taps)
  H[b, j, i] = k[b, j-i] if 0 <= j-i < 30 else 0

Strategy (per core, 64 batches):
  * The 29 taps k[1..29] per batch are computed host-side (512 x 29 values,
    negligible) and staged in the per-core DRAM input as [64, 64] f32 rows
    [krev' (29) | zeros (35)], krev'[q] = k[29-q].  k[0] is ~1e-43 in the
    reference (t is clipped at 1e-8, so t^5 underflows f32) -- the diagonal is
    left unwritten, matching the pre-zeroed output to ~1e-43 absolute.
  * The device program writes only the in-band elements with DRAM->DRAM DMAs;
    everything off-band stays zero because run_bass_kernel_spmd pre-zeroes
    ExternalOutput buffers (donated zero buffers under the PJRT/axon path --
    documented, test-covered semantics).  Flat offset of row j's band start is
    401j - 29, so two rect shapes cover the band:

      Rect BC rows  29-399 cols [j-29, j)  src q = 0..28 (fixed window)
      Rect A  rows   1- 28 cols [0, ...)   src q = 29-j+i (sliding window)

    A reads past the taps into the zero margin for i >= j, writing zeros
    (harmless), and is cut into a 3-step staircase -- rows 1-13 @ 13 cols,
    14-21 @ 21, 22-28 @ 28 -- since row j only needs cols [0, j-1].  BC puts
    the 371-row dim first in the access pattern and A puts the 64-batch dim
    first (descriptor sets on hardware are identical under any dim order;
    the leading dim rides the 16-way DMA-engine parallelism).  Work is
    split across the three DMA-capable engines -- SP: BC x16 + A[1..13] +
    A[14..21], Act: BC x21 + A[22..28], Pool: BC x14 + BC x13 -- three
    concurrent DMA pipes balanced against each engine's DMA completion
    latency (SP/Act ~1.72 us, Pool ~1.88 us).  Pool's software DGE cannot
    generate negative-stride descriptors and tops out between 6k and 8.9k
    descriptors per instruction, hence Pool takes only fixed-window BC work
    in 5194/4823-descriptor chunks.
"""

import numpy as np

B = 512
T = 400
L = 30
NCORES = 8
BPC = B // NCORES  # 64 batches per core
IW = 64            # input row width per batch (29 taps + zero margin)

_CACHE = {}


def _host_taps(theta):
    """theta [B] -> krev' [B, 29] f32: krev'[b, q] = k[b, 29-q], taps d=1..29.

    t matches the reference grid: f32 linspace(0, 30, 30000)[::1000], clipped
    at 1e-8.  Tap math in float64 then cast (diff vs the reference's f32
    device math is ~1e-7 relative).
    """
    t = np.linspace(0.0, 30.0, 30000, dtype=np.float32)[::1000]
    t = np.maximum(t, np.float32(1e-8)).astype(np.float64)  # [30]
    b = theta.astype(np.float64)[:, None]                   # [B,1]
    ebt = np.exp(-b * t)
    peak = b**6 * t**5 * ebt / 120.0
    under = b**16 * t**15 * ebt / 1307674368000.0
    k = peak - 0.167 * under                                # [B,30]
    return np.ascontiguousarray(k[:, 29:0:-1]).astype(np.float32)


def _in_map(theta_slice):
    row = np.zeros((BPC, IW), dtype=np.float32)
    row[:, :29] = _host_taps(theta_slice)
    return {"inp": row}


# BC batch split across the three DMA-capable engines.  DMA completion
# (sem update or drain) lands at proc_end + init_delay (1717 ns for SP/Act,
# 1883 for Pool), so the split equalizes proc + init_delay per engine.
# Pool's SWDGE cannot generate negative-stride descriptors, so it takes
# only BC work (strides >= 0); the sliding-window A rects go to SP/Act.
# SWDGE also fails above ~6-8k descriptors per instruction, so Pool's 27
# batches are issued as 14+13-batch DMAs (5194/4823 descriptors).
_SPLIT = [(0, 16), (16, 21), (37, 14), (51, 13)]
# A staircase: (j0, nrows, ncols) pieces; row j only needs cols [0, j-1],
# so narrower columns for lower rows (each piece stays above the 500 ns
# per-instruction floor).  SP takes the first two, Act the third.
_ASPLIT = [(1, 13, 13), (14, 8, 21), (22, 7, 28)]


def _build_nc():
    import concourse.bass as bass
    import concourse.mybir as mybir
    from concourse.ap import AP
    from contextlib import ExitStack

    f32 = mybir.dt.float32
    nc = bass.Bass()

    inp = nc.declare_dram_parameter("inp", [BPC, IW], f32, isOutput=False)
    out = nc.declare_dram_parameter("H", [BPC, T, T], f32, isOutput=True)
    in_t = inp[:].tensor
    out_t = out[:].tensor

    ctx = ExitStack()
    nc._kernel_ctx = ctx
    osem = ctx.enter_context(nc.semaphore("osem"))
    psem = ctx.enter_context(nc.semaphore("psem"))

    def bc_aps(b0, nb):
        src = AP(tensor=in_t, offset=IW * b0,
                 ap=[[0, 371], [IW, nb], [1, 29]])
        dst = AP(tensor=out_t, offset=401 * 29 - 29 + T * T * b0,
                 ap=[[401, 371], [T * T, nb], [1, 29]])
        return dst, src

    def a_aps(j0, nr, ncol):
        # rows j0..j0+nr-1, cols [0, ncol), all 64 batches (batch dim first)
        src = AP(tensor=in_t, offset=29 - j0,
                 ap=[[IW, BPC], [-1, nr], [1, ncol]])
        dst = AP(tensor=out_t, offset=T * j0,
                 ap=[[T * T, BPC], [T, nr], [1, ncol]])
        return dst, src

    # No Block: the exit all-engine barrier (drains + gather/release) costs
    # ~200 ns after the last DMA completion and is redundant here -- the SP
    # wait_ge pair observes every DMA's completion semaphore (which on
    # hardware fires when the transfer's writes land), so SP halts last and
    # the program cannot end with writes in flight.
    nc.sync.dma_start(*bc_aps(*_SPLIT[0])).then_inc(osem, 16)
    nc.sync.dma_start(*a_aps(*_ASPLIT[0])).then_inc(osem, 16)
    nc.sync.dma_start(*a_aps(*_ASPLIT[1])).then_inc(osem, 16)
    nc.scalar.dma_start(*bc_aps(*_SPLIT[1])).then_inc(osem, 16)
    nc.scalar.dma_start(*a_aps(*_ASPLIT[2])).then_inc(osem, 16)
    nc.gpsimd.dma_start(*bc_aps(*_SPLIT[2])).then_inc(psem, 16)
    nc.gpsimd.dma_start(*bc_aps(*_SPLIT[3])).then_inc(psem, 16)
    nc.sync.wait_ge(osem, 80)
    nc.sync.wait_ge(psem, 32)

    return nc


def _get_nc():
    if "nc" not in _CACHE:
        _CACHE["nc"] = _build_nc()
    return _CACHE["nc"]


def kernel(theta):
    from concourse.bass_utils import run_bass_kernel_spmd

    theta = np.asarray(theta, dtype=np.float32).reshape(B)
    in_maps = [_in_map(theta[c * BPC:(c + 1) * BPC]) for c in range(NCORES)]
    nc = _get_nc()
    res = run_bass_kernel_spmd(nc, in_maps, list(range(NCORES)))
    return np.concatenate([res.results[i]["H"] for i in range(NCORES)], axis=0)
